# revision 30
# baseline (speedup 1.0000x reference)
"""Trainium2 Bass kernel for nn_CorePartLayer.

Computes: proj = (L * z) @ U + mu  -> (B, DIM); reshaped to (B, C, 32, 32, 32)
and placed at offset 16 on each spatial axis inside a zero (B, C, 64, 64, 64)
output.

Sharding: one channel per NeuronCore (DIM = C * 32^3 and C == n_cores == 8).
Core c gets U[:, c*32768:(c+1)*32768], computes the full-batch projection for
its channel, and writes the dense 32^3 interior block. The host places the 8
channel blocks into the zero (B, C, 64, 64, 64) output (the periphery is
identically zero, exactly as the reference's zero-grid placement).

Fast path (mu == 0, the case setup_inputs produces):
  - U is rounded to bf16 on the host (the projection is a 64-term dot product;
    bf16 operand rounding keeps relative error ~2e-3, well under tolerance),
    halving the dominant HBM read traffic, and pre-swizzled to [4, 128, 4096]
    so every U-chunk DMA spans all 128 SBUF partitions (all 16 AXI ports; a
    64-partition read DMA only reaches half the ports and caps at ~250GB/s).
  - lhsT = (L*z).T is prepared host-side in bf16, duplicated to partitions
    64..128 (the PE addresses each half via an explicit tile_position), so the
    first matmul depends only on two DMAs instead of a transpose chain.
  - 4 chunk iterations, each: 1MB read (8KB lines, all issued up front on the
    sync queue so the read stream runs back-to-back) -> 16 bf16 matmuls
    (M=32, N=512, PE column tiling at partition 32j) -> 4 full-partition
    PSUM->SBUF bf16 casts split between the DVE and ACT engines -> one
    contiguous 512KB bf16 store (4KB lines) issued by the ACT engine right
    after its own cast (in-order, no cross-engine semaphore wakeup on the
    critical tail).
  - Device output layout is [chunk, 32j+b, 2*1024] so stores are fully
    contiguous; the host unscrambles to (b, d, h, w) and casts to f32.

General path (mu != 0): original f32 K=65 program (mu rides the matmul as a
ones row), writing h-rows [16,48) of the interior d-planes.
"""

from contextlib import ExitStack

import ml_dtypes
import numpy as np

import concourse.bass as bass
import concourse.tile as tile
from concourse import bacc, mybir
from concourse.bass_utils import run_bass_kernel_spmd

B = 32          # batch
NB = 64         # n_basis (contraction)
C = 8           # channels == n_cores
CORE = 32       # core cube edge
RES = 64        # output cube edge
POS = 16        # placement offset
CPD = CORE * CORE * CORE  # columns per channel = 32768
PLANE = RES * RES         # 4096 floats per padded d-plane
GROUP = 4                 # d-planes per matmul group
NCHUNK = 4                # U chunks (2 groups each) per core
F32 = mybir.dt.float32
BF16 = mybir.dt.bfloat16

_NC_CACHE = {}


def _emit_fast(ctx, tc):
    """mu == 0 specialization: bf16 U, dense interior-only output."""
    nc = tc.nc
    lhsT = nc.dram_tensor("lhsT", [2 * NB, B], BF16, kind="ExternalInput").ap()
    U = nc.dram_tensor("U", [NCHUNK, 2 * NB, GROUP * 1024], BF16,
                       kind="ExternalInput").ap()
    # bf16 output (host casts back to f32): halves write traffic; rounding
    # adds ~2e-3 relative error, total stays ~7x under tolerance.
    out = nc.dram_tensor("out", [NCHUNK, 2 * NB, 2048], BF16,
                         kind="ExternalOutput").ap()

    const = ctx.enter_context(tc.tile_pool(name="const", bufs=1))
    upool = ctx.enter_context(tc.tile_pool(name="u", bufs=NCHUNK))
    spool = ctx.enter_context(tc.tile_pool(name="st", bufs=NCHUNK))
    pmm = ctx.enter_context(tc.tile_pool(name="pmm", bufs=6, space="PSUM"))

    lhsT_t = const.tile([2 * NB, B], BF16, tag="lhsT")
    nc.sync.dma_start(lhsT_t[:, :], lhsT)

    # Reads ride the sync queue, all issued up front (bufs=NCHUNK, no reuse
    # waits) so the read stream runs back-to-back at full rate. Stores ride
    # the ACT engine's queue, with issue points placed in its in-order
    # instruction stream so the first store transfer lands just as the last
    # read drains (store packets interleaving into the read tail stretch the
    # critical read stream). Each store issue follows the ACT engine's own
    # cast of that chunk, so there is no cross-engine semaphore sleep/wakeup
    # (~2us) on the critical tail.
    u_ts = []
    for G in range(NCHUNK):
        u2 = upool.tile([2 * NB, GROUP * 1024], BF16, tag="u")
        nc.sync.dma_start(u2[:, :], U[G, :, :])
        u_ts.append(u2)

    st_ts = []
    for G in range(NCHUNK):
        u2 = u_ts[G]
        c0 = 0
        st = spool.tile([128, 2048], BF16, tag="st")
        st_ts.append(st)
        for h in range(2):
            pA = pmm.tile([128, 512], F32, tag="mm")
            pB = pmm.tile([128, 512], F32, tag="mm")
            for j in range(GROUP):
                # PSUM partition 32j+b <- proj[b, plane 8G+4h+j]
                nc.tensor.matmul(
                    pA[32 * j : 32 * j + 32, :],
                    lhsT_t[NB * h : NB * h + NB, :],
                    u2[NB * h : NB * h + NB, c0 + j * 1024 : c0 + j * 1024 + 512],
                    start=True,
                    stop=True,
                    tile_position=(NB * h, 32 * j),
                )
                nc.tensor.matmul(
                    pB[32 * j : 32 * j + 32, :],
                    lhsT_t[NB * h : NB * h + NB, :],
                    u2[
                        NB * h : NB * h + NB,
                        c0 + j * 1024 + 512 : c0 + (j + 1) * 1024,
                    ],
                    start=True,
                    stop=True,
                    tile_position=(NB * h, 32 * j),
                )
            nc.vector.tensor_copy(
                st[:, 1024 * h : 1024 * h + 512], pA[:, :]
            )
            nc.scalar.activation(
                st[:, 1024 * h + 512 : 1024 * (h + 1)],
                pB[:, :],
                mybir.ActivationFunctionType.Copy,
            )
        # Store issue points in the ACT queue: w0 after chunk 1's casts,
        # w1+w2 after chunk 2's, w3 right after chunk 3's (512KB each,
        # 4KB bf16 lines).
        if G == 1:
            # w0 rides the sync queue: its transfer queues in FIFO order
            # behind r3, starting exactly at read-drain with no read-tail
            # collision; q10's ~4.7us activation lag moves to w1, landing
            # harmlessly mid-stream.
            nc.sync.dma_start(out[0, :, :], st_ts[0][:, :])
        elif G == 2:
            nc.scalar.dma_start(out[1, :, :], st_ts[1][:, :])
            nc.scalar.dma_start(out[2, :, :], st_ts[2][:, :])
        elif G == 3:
            nc.scalar.dma_start(out[3, :, :], st_ts[3][:, :])


def _emit_general(ctx, tc):
    """General mu != 0 path: f32, K=65 (mu as a ones contraction row)."""
    nc = tc.nc
    z = nc.dram_tensor("z", [B, NB], F32, kind="ExternalInput").ap()
    Ld = nc.dram_tensor("L", [NB, 1], F32, kind="ExternalInput").ap()
    U = nc.dram_tensor("U", [NB, CPD], F32, kind="ExternalInput").ap()
    mu = nc.dram_tensor("mu", [CPD], F32, kind="ExternalInput").ap()
    out = nc.dram_tensor("out", [B, RES, PLANE], F32, kind="ExternalOutput").ap()

    const = ctx.enter_context(tc.tile_pool(name="const", bufs=1))
    upool = ctx.enter_context(tc.tile_pool(name="u", bufs=3))
    pads = ctx.enter_context(tc.tile_pool(name="pads", bufs=1))
    pzt = ctx.enter_context(tc.tile_pool(name="pzt", bufs=1, space="PSUM"))
    pmm = ctx.enter_context(tc.tile_pool(name="pmm", bufs=6, space="PSUM"))

    # --- lhsT prep: lhsT[k, b] = L[k] * z[b, k]; row NB is ones (mu row) ---
    z_t = const.tile([B, NB], F32, tag="z")
    L_t = const.tile([NB, 1], F32, tag="L")
    ones_t = const.tile([B, B], F32, tag="ones")
    id_t = const.tile([B, B], F32, tag="ident")
    lhsT = const.tile([NB + 1, B], F32, tag="lhsT")

    nc.sync.dma_start(z_t[:, :], z)
    nc.sync.dma_start(L_t[:, :], Ld)
    nc.vector.memset(ones_t[:, :], 1.0)
    nc.gpsimd.affine_select(
        id_t[:, :],
        ones_t[:, :],
        pattern=[[-1, B]],
        compare_op=mybir.AluOpType.is_equal,
        fill=0.0,
        base=0,
        channel_multiplier=1,
    )
    zTp = pzt.tile([NB, B], F32, tag="zT")
    nc.tensor.transpose(zTp[:, :], z_t[:, :], id_t[:, :])
    nc.vector.tensor_scalar(
        lhsT[0:NB, :], zTp[:, :], L_t[0:NB, :], None, mybir.AluOpType.mult
    )
    nc.vector.memset(lhsT[NB : NB + 1, :], 1.0)

    # --- trimmed padded-plane buffers (rows [16,48) of each d-plane) ---
    pwidth = CORE * RES
    NPAD = 3
    pad_ts = []
    for i in range(NPAD):
        t = pads.tile([128, pwidth], F32, tag=f"pad{i}")
        nc.vector.memset(t[:, :], 0.0)
        pad_ts.append(t)

    for g in range(CORE // GROUP):
        u_t = upool.tile([NB + 1, GROUP * 1024], F32, tag="u")
        c0 = g * GROUP * 1024
        nc.scalar.dma_start(u_t[0:NB, :], U[:, c0 : c0 + GROUP * 1024])
        nc.scalar.dma_start(u_t[NB : NB + 1, :], mu[c0 : c0 + GROUP * 1024])

        pA = pmm.tile([128, 512], F32, tag="mm")
        pB = pmm.tile([128, 512], F32, tag="mm")
        for j in range(GROUP):
            nc.tensor.matmul(
                pA[32 * j : 32 * j + 32, :],
                lhsT[:, :],
                u_t[:, j * 1024 : j * 1024 + 512],
                start=True,
                stop=True,
                tile_position=(0, 32 * j),
            )
            nc.tensor.matmul(
                pB[32 * j : 32 * j + 32, :],
                lhsT[:, :],
                u_t[:, j * 1024 + 512 : (j + 1) * 1024],
                start=True,
                stop=True,
                tile_position=(0, 32 * j),
            )

        pad_t = pad_ts[g % NPAD]
        pad3 = pad_t.rearrange("p (h w) -> p h w", w=RES)
        nc.vector.tensor_copy(
            pad3[:, 0:16, POS : POS + CORE],
            pA.rearrange("p (h w) -> p h w", w=CORE),
        )
        nc.vector.tensor_copy(
            pad3[:, 16:CORE, POS : POS + CORE],
            pB.rearrange("p (h w) -> p h w", w=CORE),
        )

        d0 = POS + GROUP * g
        f0 = POS * RES
        for j in range(GROUP):
            eng = nc.sync if j < 2 else nc.gpsimd
            eng.dma_start(
                out[:, d0 + j, f0 : f0 + pwidth],
                pad_t[32 * j : 32 * j + 32, :],
            )


def build_nc(fast=False):
    nc = bacc.Bacc(
        "TRN2",
        target_bir_lowering=False,
        debug=False,
        enable_asserts=True,
        num_devices=C,
    )
    with tile.TileContext(nc) as tc:
        with ExitStack() as ctx:
            if fast:
                _emit_fast(ctx, tc)
            else:
                _emit_general(ctx, tc)
    nc.compile()
    return nc


def make_in_maps(z, U, L, mu):
    z = np.ascontiguousarray(z, dtype=np.float32)
    L = np.ascontiguousarray(L, dtype=np.float32)
    in_maps = []
    if not np.any(np.asarray(mu)):
        lz = (L.reshape(1, NB) * z).T  # (NB, B) f32
        lhsT = np.ascontiguousarray(
            np.concatenate([lz, lz], axis=0)
        ).astype(ml_dtypes.bfloat16)  # (128, B), duplicated halves
        Ub = np.asarray(U, dtype=np.float32).astype(ml_dtypes.bfloat16)
        for c in range(C):
            Uc = Ub[:, c * CPD : (c + 1) * CPD]  # (64, 32768)
            # [G, 64h+k, f] = Uc[k, 8192G + 4096h + f]
            swiz = np.ascontiguousarray(
                Uc.reshape(NB, NCHUNK, 2, GROUP * 1024).transpose(1, 2, 0, 3)
            ).reshape(NCHUNK, 2 * NB, GROUP * 1024)
            in_maps.append({"lhsT": lhsT, "U": swiz})
    else:
        U = np.ascontiguousarray(U, dtype=np.float32)
        mu = np.ascontiguousarray(mu, dtype=np.float32)
        for c in range(C):
            in_maps.append(
                {
                    "z": z,
                    "L": L.reshape(NB, 1),
                    "U": np.ascontiguousarray(U[:, c * CPD : (c + 1) * CPD]),
                    "mu": np.ascontiguousarray(mu[c * CPD : (c + 1) * CPD]),
                }
            )
    return in_maps


def get_nc(fast):
    key = "fast" if fast else "general"
    if key not in _NC_CACHE:
        _NC_CACHE[key] = build_nc(fast=fast)
    return _NC_CACHE[key]


def decode_fast_out(arr):
    """(NCHUNK, 128, 2048) bf16 device layout -> (B, d, h, w) f32 block."""
    # [G, j, b, h, hw] with d = 8*G + 4*h + j
    a = np.asarray(arr).reshape(NCHUNK, GROUP, B, 2, 1024)
    return (
        a.transpose(2, 0, 3, 1, 4)
        .reshape(B, CORE, CORE, CORE)
        .astype(np.float32)
    )


def kernel(z, U, L, mu):
    fast = not np.any(np.asarray(mu))
    nc = get_nc(fast)
    in_maps = make_in_maps(z, U, L, mu)
    res = run_bass_kernel_spmd(nc, in_maps, core_ids=list(range(C)))
    full = np.zeros((B, C, RES, RES, RES), dtype=np.float32)
    if fast:
        for c in range(C):
            full[:, c, POS : POS + CORE, POS : POS + CORE, POS : POS + CORE] = (
                decode_fast_out(res.results[c]["out"])
            )
    else:
        for c in range(C):
            vol = np.asarray(res.results[c]["out"]).reshape(B, RES, RES, RES)
            full[:, c] = vol
    return full
